# revision 30
# baseline (speedup 1.0000x reference)
"""Bass/Trainium2 kernel for nn_Bert_coss (8-core data-parallel over batch).

Computation (per example):
  o1 = relu(X1 @ W.T + b)            [S, H]
  o2 = relu(X2 @ W.T + b)            [S, H]
  o1_doc, o2_doc = mean over S       [H]
  out = sigmoid(relu(concat(o1_doc, o2_doc) @ fd_w.T + fd_b) @ ff_w.T + ff_b)
  scores[s] = o1e[s] . o2_doc   (o1e = o1 ++ o1_doc row), s in 0..S
  att = softmax(scores); output rows 0..S-1 = att[0:S], row S = out.

Key algorithmic simplification: the reference's full [S+1,S+1] co-attention
einsum is only consumed through its last column, so only S+1 dot products
against o2_doc are needed.

Precision strategy (the kernel is HBM-bound at fp16, so inputs are fp8):
  - X1/W feed the softmax scores *per-element* (score = o1[s] . o2_doc), so
    they use float8e3 (e3m4, 4 mantissa bits) at full PE rate.
  - X2 only enters through its doc-mean (error averages down by sqrt(S)),
    so it tolerates float8e4 (e4m3) and runs DoubleRow (2 K-planes per
    instruction -> half the PE instructions).
  Weights/inputs are pre-scaled on host into the fp8 normal range; the
  PSUM eviction undoes the scale via the ACT scale operand.

X1's last V-chunk pair also rides e4m3 DoubleRow (it is appended to the
x2 stream so it shares those DMAs); its scale product matches the e3m4
part's 256 so both accumulate into one PSUM group.

Scheduling: all X DMAs are enqueued up front on the sync queue in
consumption order (X2 as paired-example mega-triggers — trigger issue
rate, not bandwidth, limits the DMA ramp), with the head parameters
last so they don't compete with the X stream for early HBM; a dozen
dummy matmuls spin the PE toward full clock during the DMA ramp; score
products run on the idle DVE so the PE-side score reduction is a single
ones-matvec; outputs leave via the sync queue's HW DGE; the sigmoid
head is issued after the last example's score matvec so its pipeline
drains under the final softmax.
"""

import sys

for _p in ("/opt/trn_rl_repo",):
    if _p not in sys.path:
        sys.path.append(_p)

import numpy as np
import ml_dtypes
from contextlib import ExitStack

import concourse.bass as bass
import concourse.tile as tile
from concourse import bacc, mybir
from concourse import bass_utils

B, S, V, H = 64, 512, 768, 256
NCORES = 8
BL = B // NCORES        # examples per core
KV = V // 128           # contraction chunks for the X1 (e3m4) matmul
KD = V // 256           # DoubleRow contraction chunks for the X2 (e4m3) matmul
MH = H // 128           # output-partition chunks of H

# host-side pre-scales to land fp8 values in the normal range
SX1 = 2.0               # X1 in e3m4 (max |x|*2 ~ 11 < 15.5)
SW1 = 128.0             # W in e3m4 (max |w|*128 ~ 12 < 15.5)
SX2 = 16.0              # X2 in e4m3 (max |x|*16 ~ 88 < 240)
SW2 = 1024.0            # W in e4m3 (max |w|*1024 ~ 94 < 240)
# X1's last two V-chunks ride e4m3 DoubleRow; their scale product must match
# the e3m4 part's SX1*SW1=256 since both accumulate into one PSUM group
SX1E = 16.0
SW1E = 16.0
KV3 = 4                 # e3m4 k-chunks of X1 (the rest is one DR pair)

F32 = mybir.dt.float32
F16 = mybir.dt.float16
E3 = mybir.dt.float8e3
E4 = mybir.dt.float8e4
AF = mybir.ActivationFunctionType
OP = mybir.AluOpType
DR = mybir.MatmulPerfMode.DoubleRow
NWARM = 14              # PE clock-ramp dummy matmuls


def _build_kernel(tc):
    nc = tc.nc
    x1t = nc.dram_tensor("x1t", [BL, 128, KV3 * S], E3, kind="ExternalInput").ap()
    # x2t carries x2 (3 DR chunks) plus x1's e4m3 DR pair per example
    x2t = nc.dram_tensor("x2t", [BL, 128, 4 * 2 * S], E4, kind="ExternalInput").ap()
    w1 = nc.dram_tensor("w1", [128, KV3 * H], E3, kind="ExternalInput").ap()
    w1e = nc.dram_tensor("w1e", [128, MH * 2 * 128], E4, kind="ExternalInput").ap()
    w2 = nc.dram_tensor("w2", [128, KD * MH * 2 * 128], E4, kind="ExternalInput").ap()
    mlp_b = nc.dram_tensor("mlp_b", [H, 1], F32, kind="ExternalInput").ap()
    fdwt = nc.dram_tensor("fdwt", [2 * H, H], F16, kind="ExternalInput").ap()
    fd_b = nc.dram_tensor("fd_b", [H, 1], F32, kind="ExternalInput").ap()
    ffwt = nc.dram_tensor("ffwt", [H, 1], F32, kind="ExternalInput").ap()
    ff_b = nc.dram_tensor("ff_b", [1, 1], F32, kind="ExternalInput").ap()
    out = nc.dram_tensor("out", [BL, S + 1], F32, kind="ExternalOutput").ap()

    with ExitStack() as ctx:
        const = ctx.enter_context(tc.tile_pool(name="const", bufs=1))

        mlpb_sb = const.tile([128, MH], F32)
        fdwt_sb = const.tile([128, 4 * H], F16)
        fdb_sb = const.tile([128, MH], F32)
        ffwt_sb = const.tile([128, MH], F32)
        ffb_sb = const.tile([1, 1], F32)
        nffb_sb = const.tile([1, 1], F32)
        ones_sb = const.tile([128, 1], F16)
        nc.vector.memset(ones_sb[:], 1.0)
        expwarm = const.tile([1, 1], F32)
        zz = const.tile([1, 1], F32)
        nc.vector.memset(zz[:], 0.0)
        # dummy Exp so the ACT table set loads during the DMA ramp instead of
        # on the end-of-kernel critical path
        nc.scalar.activation(expwarm[:], zz[:], AF.Exp, scale=0.0)
        # PE clock-ramp spin source (see NWARM below)
        dumw = const.tile([128, S + 1], E3)
        nc.vector.memset(dumw[:], 0.0)

        # weight chunks as separate tiles so the k=0 matmul only depends on
        # the first small DMA
        w1_v = w1.rearrange("p (k h) -> p k h", k=KV3)
        w2_v = w2.rearrange("p (k m) -> p k m", k=KD)
        w1_tiles = []
        for k in range(KV3):
            w1k = const.tile([128, H], E3, tag=f"w1{k}")
            w1_tiles.append(w1k)
            nc.scalar.dma_start(w1k[:], w1_v[:, k, :])
        w2_tiles = []
        for k in range(KD):
            w2k = const.tile([128, MH * 2 * 128], E4, tag=f"w2{k}")
            w2_tiles.append(w2k)
            nc.scalar.dma_start(w2k[:], w2_v[:, k, :])
        w1e_sb = const.tile([128, MH * 2 * 128], E4)
        nc.scalar.dma_start(w1e_sb[:], w1e[:, :])
        nc.scalar.dma_start(
            mlpb_sb[:].rearrange("p (m o) -> p m o", m=MH),
            mlp_b.rearrange("(m p) o -> p m o", p=128),
        )

        # doc-vector raw sums; column b*4 + kc, kc in (o1m0, o1m1, o2m0, o2m1)
        docs_all = const.tile([128, 4 * BL], F32)

        with ExitStack() as mctx:
            x1pool = mctx.enter_context(tc.tile_pool(name="x1", bufs=BL))
            x2pool = mctx.enter_context(tc.tile_pool(name="x2", bufs=1))
            o1pool = mctx.enter_context(tc.tile_pool(name="o1", bufs=2))
            o2pool = mctx.enter_context(tc.tile_pool(name="o2", bufs=2))
            dpool = mctx.enter_context(tc.tile_pool(name="docs", bufs=2))
            apool = mctx.enter_context(tc.tile_pool(name="att", bufs=3))
            ppool = mctx.enter_context(tc.tile_pool(name="prod", bufs=2))
            mm_ps = mctx.enter_context(tc.tile_pool(name="mmps", bufs=2, space="PSUM"))
            sc_ps = mctx.enter_context(tc.tile_pool(name="scps", bufs=2, space="PSUM"))
            dd_ps = mctx.enter_context(tc.tile_pool(name="ddps", bufs=2, space="PSUM"))

            # ---- all X DMAs up front on the sync queue, interleaved in
            # consumption order; X2 rides as paired-example mega-triggers
            # (trigger issue rate, not bandwidth, limits the DMA ramp)
            x1sbs = []
            for b in range(BL):
                x1sbs.append(x1pool.tile([128, KV3 * S], E3, tag="x1sb",
                                         name=f"x1sb{b}"))
            XB = 4 * 2 * S       # per-partition bytes: x2 (3 pairs) + x1 pair
            x2all = x2pool.tile([128, BL * XB], E4)
            x2sbs = [x2all[:, b * XB : (b + 1) * XB] for b in range(BL)]
            x2t_v = x2t.rearrange("b p f -> p b f")

            def _x2_dma(b0, nb):
                nc.sync.dma_start(
                    x2all[:, b0 * XB : (b0 + nb) * XB].rearrange(
                        "p (b f) -> p b f", b=nb),
                    x2t_v[:, b0 : b0 + nb, :],
                )

            # x1 b0 in 2-k sub-chunks so the k=0 matmul starts early
            for c in range(2):
                nc.sync.dma_start(
                    x1sbs[0][:, c * 2 * S : (c + 1) * 2 * S],
                    x1t[0][:, c * 2 * S : (c + 1) * 2 * S],
                )
            _x2_dma(0, 1)
            for c in range(2):
                nc.sync.dma_start(
                    x1sbs[1][:, c * 2 * S : (c + 1) * 2 * S],
                    x1t[1][:, c * 2 * S : (c + 1) * 2 * S],
                )
            _x2_dma(1, 2)
            for c in range(2):
                nc.sync.dma_start(
                    x1sbs[2][:, c * 2 * S : (c + 1) * 2 * S],
                    x1t[2][:, c * 2 * S : (c + 1) * 2 * S],
                )
            nc.sync.dma_start(x1sbs[3][:], x1t[3])
            _x2_dma(3, 2)
            nc.sync.dma_start(x1sbs[4][:], x1t[4])
            nc.sync.dma_start(x1sbs[5][:], x1t[5])
            _x2_dma(5, 3)
            nc.sync.dma_start(x1sbs[6][:], x1t[6])
            nc.sync.dma_start(x1sbs[7][:], x1t[7])
            # head parameters last: needed only at ~the end of the kernel,
            # so they must not compete with the X stream for early HBM
            nc.sync.dma_start(
                fdwt_sb[:].rearrange("p (k h) -> p k h", k=4),
                fdwt.rearrange("(k p) h -> p k h", p=128),
            )
            nc.sync.dma_start(
                fdb_sb[:].rearrange("p (m o) -> p m o", m=MH),
                fd_b.rearrange("(m p) o -> p m o", p=128),
            )
            nc.sync.dma_start(
                ffwt_sb[:].rearrange("p (m o) -> p m o", m=MH),
                ffwt.rearrange("(m p) o -> p m o", p=128),
            )
            nc.sync.dma_start(ffb_sb[:], ff_b[:, :])
            nc.vector.tensor_scalar_mul(nffb_sb[:], ffb_sb[:], -1.0)

            # PE clock-ramp spin: dummy matmuls with no DMA deps keep the PE
            # array busy through the preamble + DMA ramp so the first real
            # matmuls run at full clock instead of the cold half-rate pstate
            for _ in range(NWARM):
                dps = sc_ps.tile([1, S], F32, name="ssc")
                nc.tensor.matmul(dps[:], dumw[:, 0:1], dumw[:, 1 : S + 1],
                                 start=True, stop=True)

            def do_scores(b, o1T, dsc, dscf, after=None, final=False):
                # per-s score products on the idle DVE so the PE-side
                # reduction is a single ones-matvec:
                #   prod[p,s] = sum_m o1T[p,m,s] * o2d[p,m]
                p0 = ppool.tile([128, S], F16, name="p0")
                nc.vector.tensor_scalar_mul(p0[:], o1T[:, 0:S], dscf[:, 2:3])
                prod = ppool.tile([128, S], F16, name="prod")
                nc.vector.scalar_tensor_tensor(
                    prod[:], o1T[:, S : 2 * S], dscf[:, 3:4], p0[:],
                    op0=OP.mult, op1=OP.add,
                )
                ssc = sc_ps.tile([1, S], F32)
                mm = nc.tensor.matmul(ssc[:], ones_sb[:], prod[:],
                                      start=True, stop=True)
                if after is not None:
                    # keep PE from stalling: order the score matvec after
                    # the current example's dense matmuls (order-only edge)
                    tile.add_dep_helper(
                        mm.ins, after.ins, sync=False,
                        reason="pipeline scores behind next example's mlp",
                    )
                sdd = dd_ps.tile([1, 1], F32)
                for hk in range(MH):
                    mm = nc.tensor.matmul(
                        sdd[:],
                        dsc[:, 2 + hk : 3 + hk],
                        dsc[:, hk : hk + 1],
                        start=(hk == 0),
                        stop=(hk == MH - 1),
                    )
                    if after is not None:
                        tile.add_dep_helper(
                            mm.ins, after.ins, sync=False,
                            reason="pipeline scores behind next example's mlp",
                        )
                # softmax on partition 0, straight from PSUM; no max-
                # subtraction (scores are O(25), far inside fp32 exp range)
                att = apool.tile([1, S], F32)
                s1 = apool.tile([1, 1], F32, name="s1")
                nc.scalar.activation(att[:], ssc[:], AF.Exp, accum_out=s1[:])
                edd = apool.tile([1, 1], F32, name="edd")
                nc.scalar.activation(edd[:], sdd[:], AF.Exp)
                stot = apool.tile([1, 1], F32, name="stot")
                nc.vector.tensor_add(stot[:], s1[:], edd[:])
                rs = apool.tile([1, 1], F32, name="rs")
                nc.vector.reciprocal(rs[:], stot[:])
                nc.vector.tensor_scalar_mul(att[:], att[:], rs[:])
                # HWDGE on the sync queue — idle once the upfront X
                # triggers are enqueued, and completes faster than SWDGE
                nc.sync.dma_start(out[b : b + 1, 0:S], att[:])

            def x1_block(b, o1T, x1sb, x2sb):
                # X1 matmuls: 8 e3m4 rate-1.0 + 2 e4m3 DoubleRow (the
                # last V-chunk pair), all one PSUM accumulation group
                pss = [
                    mm_ps.tile([128, S], F32, tag=f"ps{m}", name=f"ps{m}")
                    for m in range(MH)
                ]
                for k in range(KV3):
                    rhs = x1sb[:, k * S : (k + 1) * S]
                    for m in range(MH):
                        nc.tensor.matmul(
                            pss[m][:],
                            w1_tiles[k][:, m * 128 : (m + 1) * 128],
                            rhs,
                            start=(k == 0),
                            stop=False,
                        )
                x1e_v = x2sb[:, 3 * 2 * S :].rearrange(
                    "p (i s) -> p i s", i=2)
                w1e_v = w1e_sb[:].rearrange("p (m i c) -> p m i c", m=MH, i=2)
                for m in range(MH):
                    mm = nc.tensor.matmul(
                        pss[m][:],
                        w1e_v[:, m, :, :],
                        x1e_v,
                        start=False,
                        stop=True,
                        perf_mode=DR,
                    )
                for m in range(MH):
                    nc.scalar.activation(
                        o1T[:, m * S : (m + 1) * S],
                        pss[m][:],
                        AF.Relu,
                        bias=mlpb_sb[:, m : m + 1],
                        scale=1.0 / (SX1 * SW1),
                        accum_out=docs_all[:, b * 4 + m : b * 4 + m + 1],
                    )
                return mm

            def x2_block(b, x2sb):
                # X2 matmuls (e4m3 DoubleRow): 6 of [128x2x128] @ [128x2x512]
                ps2 = [
                    mm_ps.tile([128, S], F32, tag=f"ps{m}", name=f"q{m}")
                    for m in range(MH)
                ]
                x2v = x2sb[:, : KD * 2 * S].rearrange(
                    "p (k i s) -> p k i s", k=KD, i=2)
                last_mm = None
                for kd in range(KD):
                    rhs = x2v[:, kd, :, :]
                    for m in range(MH):
                        w2v = w2_tiles[kd][:].rearrange(
                            "p (m i c) -> p m i c", m=MH, i=2
                        )
                        last_mm = nc.tensor.matmul(
                            ps2[m][:],
                            w2v[:, m, :, :],
                            rhs,
                            start=(kd == 0),
                            stop=(kd == KD - 1),
                            perf_mode=DR,
                        )
                for m in range(MH):
                    o2scr = o2pool.tile([128, S], F16)
                    nc.scalar.activation(
                        o2scr[:],
                        ps2[m][:],
                        AF.Relu,
                        bias=mlpb_sb[:, m : m + 1],
                        scale=1.0 / (SX2 * SW2),
                        accum_out=docs_all[:, b * 4 + 2 + m : b * 4 + 2 + m + 1],
                    )
                return last_mm

            prev = None
            for b in range(BL):
                o1T = o1pool.tile([128, MH * S], F16)
                x1sb, x2sb = x1sbs[b], x2sbs[b]

                if b < BL - 1:
                    x1_block(b, o1T, x1sb, x2sb)
                    last_mm = x2_block(b, x2sb)
                    if prev is not None:
                        do_scores(*prev, after=last_mm)
                    # per-example scaled docs: [o1d0, o1d1, o2d0, o2d1]
                    dsc = dpool.tile([128, 4], F16)
                    nc.vector.tensor_scalar_mul(
                        dsc[:], docs_all[:, b * 4 : b * 4 + 4], 1.0 / S
                    )
                    dscf = dpool.tile([128, 4], F32, name="dscf")
                    nc.vector.tensor_scalar_mul(
                        dscf[:], docs_all[:, b * 4 : b * 4 + 4], 1.0 / S
                    )
                else:
                    # last example: X2 first so its o2-doc evictions and doc
                    # scaling finish during the X1 block — the final score
                    # chain then starts right after the first o1 eviction,
                    # shortening the serial tail
                    x2_block(b, x2sb)
                    dscf = dpool.tile([128, 4], F32, name="dscf")
                    nc.vector.tensor_scalar_mul(
                        dscf[:, 2:4],
                        docs_all[:, b * 4 + 2 : b * 4 + 4], 1.0 / S
                    )
                    last_mm = x1_block(b, o1T, x1sb, x2sb)
                    if prev is not None:
                        do_scores(*prev, after=last_mm)
                    nc.vector.tensor_scalar_mul(
                        dscf[:, 0:2], docs_all[:, b * 4 : b * 4 + 2], 1.0 / S
                    )
                    dsc = dpool.tile([128, 4], F16)
                    nc.vector.tensor_scalar_mul(
                        dsc[:], docs_all[:, b * 4 : b * 4 + 4], 1.0 / S
                    )
                prev = (b, o1T, dsc, dscf)

            do_scores(*prev, final=True)

            # ---- head (batched), issued right after the last example's
            # score matvec so its pipeline drains under the score softmax;
            # PSUM comes from the mm/dd rings (same tile names -> same rings)
            hpool = mctx.enter_context(tc.tile_pool(name="head", bufs=1))
            docs_sc = hpool.tile([128, 4 * BL], F16)
            nc.vector.tensor_scalar_mul(docs_sc[:], docs_all[:], 1.0 / S)
            docs_v = docs_sc[:].rearrange("p (b k) -> p k b", k=4)

            h_sb = hpool.tile([128, MH * BL], F32)
            for m in range(MH):
                ph = mm_ps.tile([128, S], F32, tag=f"ps{m}", name=f"ps{m}")
                for kc in range(4):
                    nc.tensor.matmul(
                        ph[:, 0:BL],
                        fdwt_sb[:, kc * H + m * 128 : kc * H + (m + 1) * 128],
                        docs_v[:, kc, :],
                        start=(kc == 0),
                        stop=(kc == 3),
                    )
                nc.scalar.activation(
                    h_sb[:, m * BL : (m + 1) * BL],
                    ph[:, 0:BL],
                    AF.Relu,
                    bias=fdb_sb[:, m : m + 1],
                )
            po = dd_ps.tile([1, BL], F32, name="sdd")
            for m in range(MH):
                nc.tensor.matmul(
                    po[:],
                    ffwt_sb[:, m : m + 1],
                    h_sb[:, m * BL : (m + 1) * BL],
                    start=(m == 0),
                    stop=(m == MH - 1),
                )
            # sigmoid(x) = 1/(1+exp(-x)) — stays in the Exp table set
            sig_row = hpool.tile([1, BL], F32)
            nc.scalar.activation(sig_row[:], po[:], AF.Exp,
                                 bias=nffb_sb[0:1, 0:1], scale=-1.0)
            nc.vector.tensor_scalar_add(sig_row[:], sig_row[:], 1.0)
            nc.vector.reciprocal(sig_row[:], sig_row[:])
            # final output column: out[:, S] = sigmoid head values
            nc.sync.dma_start(
                out[:, S : S + 1],
                sig_row[0:1, :].rearrange("o (b s) -> o b s", b=BL),
            )


_NC_CACHE = None


def _get_nc():
    global _NC_CACHE
    if _NC_CACHE is None:
        nc = bacc.Bacc("TRN2", target_bir_lowering=False, debug=False,
                       num_devices=NCORES)
        with tile.TileContext(nc) as tc:
            _build_kernel(tc)
        nc.compile()
        _NC_CACHE = nc
    return _NC_CACHE


def kernel(output_1, output_2, mlp_w, mlp_b, fd_w, fd_b, ff_w, ff_b):
    output_1 = np.asarray(output_1, dtype=np.float32)
    output_2 = np.asarray(output_2, dtype=np.float32)
    mlp_w = np.asarray(mlp_w, dtype=np.float32)
    mlp_b = np.asarray(mlp_b, dtype=np.float32)
    fd_w = np.asarray(fd_w, dtype=np.float32)
    fd_b = np.asarray(fd_b, dtype=np.float32)
    ff_w = np.asarray(ff_w, dtype=np.float32)
    ff_b = np.asarray(ff_b, dtype=np.float32)

    # shard over batch; pre-transpose so V lands on partitions with each
    # partition's free row contiguous in HBM
    # x1 e3m4 part (V-chunks 0..3): [c,b,p,k,s] = X1[c*BL+b, s, k*128+p]*SX1
    x1q = np.ascontiguousarray(
        output_1[:, :, : KV3 * 128]
        .reshape(NCORES, BL, S, KV3, 128).transpose(0, 1, 4, 3, 2)
        * SX1
    ).astype(ml_dtypes.float8_e3m4).reshape(NCORES, BL, 128, KV3 * S)
    # x1 e4m3 DR pair (V-chunks 4,5): [c,b,p,i,s]
    x1e = np.ascontiguousarray(
        output_1[:, :, KV3 * 128 :]
        .reshape(NCORES, BL, S, 2, 128).transpose(0, 1, 4, 3, 2)
        * SX1E
    ).astype(ml_dtypes.float8_e4m3).reshape(NCORES, BL, 128, 2 * S)
    # x2[c,b,p,kd,i,s] = X2[c*BL+b, s, kd*256+i*128+p] * SX2, e4m3;
    # x1's e4m3 pair is appended per example so it rides the same DMAs
    x2q = np.ascontiguousarray(
        output_2.reshape(NCORES, BL, S, KD, 2, 128).transpose(0, 1, 5, 3, 4, 2)
        * SX2
    ).astype(ml_dtypes.float8_e4m3).reshape(NCORES, BL, 128, KD * 2 * S)
    x2q = np.ascontiguousarray(np.concatenate([x2q, x1e], axis=3))

    wt = np.ascontiguousarray(mlp_w.T)                    # [V, H] f32
    # w1[p,k,h] = wt[k*128+p, h] * SW1, e3m4 (V rows 0..511)
    w1q = np.ascontiguousarray(
        wt[: KV3 * 128].reshape(KV3, 128, H).transpose(1, 0, 2) * SW1
    ).astype(ml_dtypes.float8_e3m4).reshape(128, KV3 * H)
    # w1e[p,m,i,c] = wt[512 + i*128+p, m*128+c] * SW1E, e4m3
    w1eq = np.ascontiguousarray(
        wt[KV3 * 128 :].reshape(2, 128, MH, 128).transpose(1, 2, 0, 3) * SW1E
    ).astype(ml_dtypes.float8_e4m3).reshape(128, MH * 2 * 128)
    # w2[p,kd,m,i,c] = wt[kd*256+i*128+p, m*128+c] * SW2, e4m3
    w2q = np.ascontiguousarray(
        wt.reshape(KD, 2, 128, MH, 128).transpose(2, 0, 3, 1, 4) * SW2
    ).astype(ml_dtypes.float8_e4m3).reshape(128, KD * MH * 2 * 128)

    mlpb = np.ascontiguousarray(mlp_b.reshape(H, 1))
    fdwt = np.ascontiguousarray(fd_w.T).astype(np.float16)  # [2H, H]
    fdb = np.ascontiguousarray(fd_b.reshape(H, 1))
    ffwt = np.ascontiguousarray(ff_w.T)                   # [H, 1]
    ffb = np.ascontiguousarray(ff_b.reshape(1, 1))

    in_maps = [
        dict(x1t=x1q[c], x2t=x2q[c], w1=w1q, w1e=w1eq, w2=w2q, mlp_b=mlpb,
             fdwt=fdwt, fd_b=fdb, ffwt=ffwt, ff_b=ffb)
        for c in range(NCORES)
    ]
    global _LAST_IN_MAPS
    _LAST_IN_MAPS = in_maps
    nc = _get_nc()
    res = bass_utils.run_bass_kernel_spmd(nc, in_maps, core_ids=list(range(NCORES)))
    att = np.concatenate([res.results[c]["out"] for c in range(NCORES)], axis=0)
    return np.ascontiguousarray(att.T)  # [S+1, B]


# revision 31
# speedup vs baseline: 1.1324x; 1.1324x over previous
"""Bass/Trainium2 kernel for nn_Bert_coss (8-core data-parallel over batch).

Computation (per example):
  o1 = relu(X1 @ W.T + b)            [S, H]
  o2 = relu(X2 @ W.T + b)            [S, H]
  o1_doc, o2_doc = mean over S       [H]
  out = sigmoid(relu(concat(o1_doc, o2_doc) @ fd_w.T + fd_b) @ ff_w.T + ff_b)
  scores[s] = o1e[s] . o2_doc   (o1e = o1 ++ o1_doc row), s in 0..S
  att = softmax(scores); output rows 0..S-1 = att[0:S], row S = out.

Key algorithmic simplification: the reference's full [S+1,S+1] co-attention
einsum is only consumed through its last column, so only S+1 dot products
against o2_doc are needed.

Precision strategy (the kernel is HBM-bound at fp16, so inputs are fp8):
  - X1/W feed the softmax scores *per-element* (score = o1[s] . o2_doc), so
    they use float8e3 (e3m4, 4 mantissa bits) at full PE rate.
  - X2 only enters through its doc-mean (error averages down by sqrt(S)),
    so it tolerates float8e4 (e4m3) and runs DoubleRow (2 K-planes per
    instruction -> half the PE instructions).
  Weights/inputs are pre-scaled on host into the fp8 normal range; the
  PSUM eviction undoes the scale via the ACT scale operand.

X1's last V-chunk pair also rides e4m3 DoubleRow (it is appended to the
x2 stream so it shares those DMAs); its scale product matches the e3m4
part's 256 so both accumulate into one PSUM group.

Scheduling: all X DMAs are enqueued up front on the sync queue in
consumption order (X2 as paired-example mega-triggers — trigger issue
rate, not bandwidth, limits the DMA ramp), with the head parameters
last so they don't compete with the X stream for early HBM; a dozen
dummy matmuls spin the PE toward full clock during the DMA ramp; score
products run on the idle DVE so the PE-side score reduction is a single
ones-matvec; outputs leave via the sync queue's HW DGE; the sigmoid
head is issued after the last example's score matvec so its pipeline
drains under the final softmax.
"""

import sys

for _p in ("/opt/trn_rl_repo",):
    if _p not in sys.path:
        sys.path.append(_p)

import numpy as np
import ml_dtypes
from contextlib import ExitStack

import concourse.bass as bass
import concourse.tile as tile
from concourse import bacc, mybir
from concourse import bass_utils

B, S, V, H = 64, 512, 768, 256
NCORES = 8
BL = B // NCORES        # examples per core
KV = V // 128           # contraction chunks for the X1 (e3m4) matmul
KD = V // 256           # DoubleRow contraction chunks for the X2 (e4m3) matmul
MH = H // 128           # output-partition chunks of H

# host-side pre-scales to land fp8 values in the normal range
SX1 = 2.0               # X1 in e3m4 (max |x|*2 ~ 11 < 15.5)
SW1 = 128.0             # W in e3m4 (max |w|*128 ~ 12 < 15.5)
SX2 = 16.0              # X2 in e4m3 (max |x|*16 ~ 88 < 240)
SW2 = 1024.0            # W in e4m3 (max |w|*1024 ~ 94 < 240)
# X1's last two V-chunks ride e4m3 DoubleRow; their scale product must match
# the e3m4 part's SX1*SW1=256 since both accumulate into one PSUM group
SX1E = 16.0
SW1E = 16.0
KV3 = 4                 # e3m4 k-chunks of X1 (the rest is one DR pair)

F32 = mybir.dt.float32
F16 = mybir.dt.float16
E3 = mybir.dt.float8e3
E4 = mybir.dt.float8e4
AF = mybir.ActivationFunctionType
OP = mybir.AluOpType
DR = mybir.MatmulPerfMode.DoubleRow
NWARM = 14              # PE clock-ramp dummy matmuls


def _build_kernel(tc):
    nc = tc.nc
    x1t = nc.dram_tensor("x1t", [BL, 128, KV3 * S], E3, kind="ExternalInput").ap()
    # x2t carries x2 (3 DR chunks) plus x1's e4m3 DR pair per example
    x2t = nc.dram_tensor("x2t", [BL, 128, 4 * 2 * S], E4, kind="ExternalInput").ap()
    w1 = nc.dram_tensor("w1", [128, KV3 * H], E3, kind="ExternalInput").ap()
    w1e = nc.dram_tensor("w1e", [128, MH * 2 * 128], E4, kind="ExternalInput").ap()
    w2 = nc.dram_tensor("w2", [128, KD * MH * 2 * 128], E4, kind="ExternalInput").ap()
    mlp_b = nc.dram_tensor("mlp_b", [H, 1], F32, kind="ExternalInput").ap()
    fdwt = nc.dram_tensor("fdwt", [2 * H, H], F16, kind="ExternalInput").ap()
    fd_b = nc.dram_tensor("fd_b", [H, 1], F32, kind="ExternalInput").ap()
    ffwt = nc.dram_tensor("ffwt", [H, 1], F32, kind="ExternalInput").ap()
    ff_b = nc.dram_tensor("ff_b", [1, 1], F32, kind="ExternalInput").ap()
    out = nc.dram_tensor("out", [BL, S + 1], F32, kind="ExternalOutput").ap()

    with ExitStack() as ctx:
        const = ctx.enter_context(tc.tile_pool(name="const", bufs=1))

        mlpb_sb = const.tile([128, MH], F32)
        fdwt_sb = const.tile([128, 4 * H], F16)
        fdb_sb = const.tile([128, MH], F32)
        ffwt_sb = const.tile([128, MH], F32)
        ffb_sb = const.tile([1, 1], F32)
        nffb_sb = const.tile([1, 1], F32)
        ones_sb = const.tile([128, 1], F16)
        nc.vector.memset(ones_sb[:], 1.0)
        expwarm = const.tile([1, 1], F32)
        zz = const.tile([1, 1], F32)
        nc.vector.memset(zz[:], 0.0)
        # dummy Exp so the ACT table set loads during the DMA ramp instead of
        # on the end-of-kernel critical path
        nc.scalar.activation(expwarm[:], zz[:], AF.Exp, scale=0.0)
        # PE clock-ramp spin source (see NWARM below)
        dumw = const.tile([128, S + 1], E3)
        nc.vector.memset(dumw[:], 0.0)

        # weight chunks as separate tiles so the k=0 matmul only depends on
        # the first small DMA
        w1_v = w1.rearrange("p (k h) -> p k h", k=KV3)
        w2_v = w2.rearrange("p (k m) -> p k m", k=KD)
        w1_tiles = []
        for k in range(KV3):
            w1k = const.tile([128, H], E3, tag=f"w1{k}")
            w1_tiles.append(w1k)
            nc.scalar.dma_start(w1k[:], w1_v[:, k, :])
        w2_tiles = []
        for k in range(KD):
            w2k = const.tile([128, MH * 2 * 128], E4, tag=f"w2{k}")
            w2_tiles.append(w2k)
            nc.scalar.dma_start(w2k[:], w2_v[:, k, :])
        w1e_sb = const.tile([128, MH * 2 * 128], E4)
        nc.scalar.dma_start(w1e_sb[:], w1e[:, :])
        nc.scalar.dma_start(
            mlpb_sb[:].rearrange("p (m o) -> p m o", m=MH),
            mlp_b.rearrange("(m p) o -> p m o", p=128),
        )

        # doc-vector raw sums; column b*4 + kc, kc in (o1m0, o1m1, o2m0, o2m1)
        docs_all = const.tile([128, 4 * BL], F32)

        with ExitStack() as mctx:
            x1pool = mctx.enter_context(tc.tile_pool(name="x1", bufs=BL))
            x2pool = mctx.enter_context(tc.tile_pool(name="x2", bufs=1))
            o1pool = mctx.enter_context(tc.tile_pool(name="o1", bufs=2))
            o2pool = mctx.enter_context(tc.tile_pool(name="o2", bufs=2))
            dpool = mctx.enter_context(tc.tile_pool(name="docs", bufs=2))
            apool = mctx.enter_context(tc.tile_pool(name="att", bufs=3))
            ppool = mctx.enter_context(tc.tile_pool(name="prod", bufs=2))
            mm_ps = mctx.enter_context(tc.tile_pool(name="mmps", bufs=2, space="PSUM"))
            sc_ps = mctx.enter_context(tc.tile_pool(name="scps", bufs=2, space="PSUM"))
            dd_ps = mctx.enter_context(tc.tile_pool(name="ddps", bufs=2, space="PSUM"))

            # ---- all X DMAs up front on the sync queue, interleaved in
            # consumption order; X2 rides as paired-example mega-triggers
            # (trigger issue rate, not bandwidth, limits the DMA ramp)
            x1sbs = []
            for b in range(BL):
                x1sbs.append(x1pool.tile([128, KV3 * S], E3, tag="x1sb",
                                         name=f"x1sb{b}"))
            XB = 4 * 2 * S       # per-partition bytes: x2 (3 pairs) + x1 pair
            x2all = x2pool.tile([128, BL * XB], E4)
            x2sbs = [x2all[:, b * XB : (b + 1) * XB] for b in range(BL)]
            x2t_v = x2t.rearrange("b p f -> p b f")

            def _x2_dma(b0, nb):
                nc.sync.dma_start(
                    x2all[:, b0 * XB : (b0 + nb) * XB].rearrange(
                        "p (b f) -> p b f", b=nb),
                    x2t_v[:, b0 : b0 + nb, :],
                )

            # x1 b0 in 2-k sub-chunks so the k=0 matmul starts early
            for c in range(2):
                nc.sync.dma_start(
                    x1sbs[0][:, c * 2 * S : (c + 1) * 2 * S],
                    x1t[0][:, c * 2 * S : (c + 1) * 2 * S],
                )
            _x2_dma(0, 1)
            for c in range(2):
                nc.sync.dma_start(
                    x1sbs[1][:, c * 2 * S : (c + 1) * 2 * S],
                    x1t[1][:, c * 2 * S : (c + 1) * 2 * S],
                )
            _x2_dma(1, 2)
            for c in range(2):
                nc.sync.dma_start(
                    x1sbs[2][:, c * 2 * S : (c + 1) * 2 * S],
                    x1t[2][:, c * 2 * S : (c + 1) * 2 * S],
                )
            nc.sync.dma_start(x1sbs[3][:], x1t[3])
            _x2_dma(3, 2)
            nc.sync.dma_start(x1sbs[4][:], x1t[4])
            nc.sync.dma_start(x1sbs[5][:], x1t[5])
            _x2_dma(5, 3)
            nc.sync.dma_start(x1sbs[6][:], x1t[6])
            nc.sync.dma_start(x1sbs[7][:], x1t[7])
            # head parameters last: needed only at ~the end of the kernel,
            # so they must not compete with the X stream for early HBM
            nc.sync.dma_start(
                fdwt_sb[:].rearrange("p (k h) -> p k h", k=4),
                fdwt.rearrange("(k p) h -> p k h", p=128),
            )
            nc.sync.dma_start(
                fdb_sb[:].rearrange("p (m o) -> p m o", m=MH),
                fd_b.rearrange("(m p) o -> p m o", p=128),
            )
            nc.sync.dma_start(
                ffwt_sb[:].rearrange("p (m o) -> p m o", m=MH),
                ffwt.rearrange("(m p) o -> p m o", p=128),
            )
            nc.sync.dma_start(ffb_sb[:], ff_b[:, :])
            nc.vector.tensor_scalar_mul(nffb_sb[:], ffb_sb[:], -1.0)

            # PE clock-ramp spin: dummy matmuls with no DMA deps keep the PE
            # array busy through the preamble + DMA ramp so the first real
            # matmuls run at full clock instead of the cold half-rate pstate
            for _ in range(NWARM):
                dps = sc_ps.tile([1, S], F32, name="ssc")
                nc.tensor.matmul(dps[:], dumw[:, 0:1], dumw[:, 1 : S + 1],
                                 start=True, stop=True)

            def do_scores(b, o1T, dsc, dscf, after=None, final=False):
                # per-s score products on the idle DVE so the PE-side
                # reduction is a single ones-matvec:
                #   prod[p,s] = sum_m o1T[p,m,s] * o2d[p,m]
                p0 = ppool.tile([128, S], F16, name="p0")
                nc.vector.tensor_scalar_mul(p0[:], o1T[:, 0:S], dscf[:, 2:3])
                prod = ppool.tile([128, S], F16, name="prod")
                nc.vector.scalar_tensor_tensor(
                    prod[:], o1T[:, S : 2 * S], dscf[:, 3:4], p0[:],
                    op0=OP.mult, op1=OP.add,
                )
                ssc = sc_ps.tile([1, S], F32)
                mm = nc.tensor.matmul(ssc[:], ones_sb[:], prod[:],
                                      start=True, stop=True)
                if after is not None:
                    # keep PE from stalling: order the score matvec after
                    # the current example's dense matmuls (order-only edge)
                    tile.add_dep_helper(
                        mm.ins, after.ins, sync=False,
                        reason="pipeline scores behind next example's mlp",
                    )
                sdd = dd_ps.tile([1, 1], F32)
                for hk in range(MH):
                    mm = nc.tensor.matmul(
                        sdd[:],
                        dsc[:, 2 + hk : 3 + hk],
                        dsc[:, hk : hk + 1],
                        start=(hk == 0),
                        stop=(hk == MH - 1),
                    )
                    if after is not None:
                        tile.add_dep_helper(
                            mm.ins, after.ins, sync=False,
                            reason="pipeline scores behind next example's mlp",
                        )
                # softmax on partition 0, straight from PSUM; no max-
                # subtraction (scores are O(25), far inside fp32 exp range)
                att = apool.tile([1, S], F32)
                s1 = apool.tile([1, 1], F32, name="s1")
                nc.scalar.activation(att[:], ssc[:], AF.Exp, accum_out=s1[:])
                edd = apool.tile([1, 1], F32, name="edd")
                nc.scalar.activation(edd[:], sdd[:], AF.Exp)
                stot = apool.tile([1, 1], F32, name="stot")
                nc.vector.tensor_add(stot[:], s1[:], edd[:])
                rs = apool.tile([1, 1], F32, name="rs")
                nc.vector.reciprocal(rs[:], stot[:])
                nc.vector.tensor_scalar_mul(att[:], att[:], rs[:])
                # HWDGE on the sync queue — idle once the upfront X
                # triggers are enqueued, and completes faster than SWDGE
                nc.sync.dma_start(out[b : b + 1, 0:S], att[:])

            def x1_block(b, o1T, x1sb, x2sb):
                # X1 matmuls: 8 e3m4 rate-1.0 + 2 e4m3 DoubleRow (the
                # last V-chunk pair), all one PSUM accumulation group
                pss = [
                    mm_ps.tile([128, S], F32, tag=f"ps{m}", name=f"ps{m}")
                    for m in range(MH)
                ]
                for k in range(KV3):
                    rhs = x1sb[:, k * S : (k + 1) * S]
                    for m in range(MH):
                        nc.tensor.matmul(
                            pss[m][:],
                            w1_tiles[k][:, m * 128 : (m + 1) * 128],
                            rhs,
                            start=(k == 0),
                            stop=False,
                        )
                x1e_v = x2sb[:, 3 * 2 * S :].rearrange(
                    "p (i s) -> p i s", i=2)
                w1e_v = w1e_sb[:].rearrange("p (m i c) -> p m i c", m=MH, i=2)
                for m in range(MH):
                    mm = nc.tensor.matmul(
                        pss[m][:],
                        w1e_v[:, m, :, :],
                        x1e_v,
                        start=False,
                        stop=True,
                        perf_mode=DR,
                    )
                for m in range(MH):
                    nc.scalar.activation(
                        o1T[:, m * S : (m + 1) * S],
                        pss[m][:],
                        AF.Relu,
                        bias=mlpb_sb[:, m : m + 1],
                        scale=1.0 / (SX1 * SW1),
                        accum_out=docs_all[:, b * 4 + m : b * 4 + m + 1],
                    )
                return mm

            def x2_block(b, x2sb):
                # X2 matmuls (e4m3 DoubleRow): 6 of [128x2x128] @ [128x2x512]
                ps2 = [
                    mm_ps.tile([128, S], F32, tag=f"ps{m}", name=f"q{m}")
                    for m in range(MH)
                ]
                x2v = x2sb[:, : KD * 2 * S].rearrange(
                    "p (k i s) -> p k i s", k=KD, i=2)
                last_mm = None
                for kd in range(KD):
                    rhs = x2v[:, kd, :, :]
                    for m in range(MH):
                        w2v = w2_tiles[kd][:].rearrange(
                            "p (m i c) -> p m i c", m=MH, i=2
                        )
                        last_mm = nc.tensor.matmul(
                            ps2[m][:],
                            w2v[:, m, :, :],
                            rhs,
                            start=(kd == 0),
                            stop=(kd == KD - 1),
                            perf_mode=DR,
                        )
                for m in range(MH):
                    o2scr = o2pool.tile([128, S], F16)
                    nc.scalar.activation(
                        o2scr[:],
                        ps2[m][:],
                        AF.Relu,
                        bias=mlpb_sb[:, m : m + 1],
                        scale=1.0 / (SX2 * SW2),
                        accum_out=docs_all[:, b * 4 + 2 + m : b * 4 + 2 + m + 1],
                    )
                return last_mm

            prev = None
            for b in range(BL):
                o1T = o1pool.tile([128, MH * S], F16)
                x1sb, x2sb = x1sbs[b], x2sbs[b]

                x1_block(b, o1T, x1sb, x2sb)
                last_mm = x2_block(b, x2sb)
                if prev is not None:
                    do_scores(*prev, after=last_mm)
                # per-example scaled docs: [o1d0, o1d1, o2d0, o2d1]
                dsc = dpool.tile([128, 4], F16)
                nc.vector.tensor_scalar_mul(
                    dsc[:], docs_all[:, b * 4 : b * 4 + 4], 1.0 / S
                )
                dscf = dpool.tile([128, 4], F32, name="dscf")
                nc.vector.tensor_scalar_mul(
                    dscf[:], docs_all[:, b * 4 : b * 4 + 4], 1.0 / S
                )
                prev = (b, o1T, dsc, dscf)

            do_scores(*prev, final=True)

            # ---- head (batched), issued right after the last example's
            # score matvec so its pipeline drains under the score softmax;
            # PSUM comes from the mm/dd rings (same tile names -> same rings)
            hpool = mctx.enter_context(tc.tile_pool(name="head", bufs=1))
            docs_sc = hpool.tile([128, 4 * BL], F16)
            nc.vector.tensor_scalar_mul(docs_sc[:], docs_all[:], 1.0 / S)
            docs_v = docs_sc[:].rearrange("p (b k) -> p k b", k=4)

            h_sb = hpool.tile([128, MH * BL], F32)
            for m in range(MH):
                ph = mm_ps.tile([128, S], F32, tag=f"ps{m}", name=f"ps{m}")
                for kc in range(4):
                    nc.tensor.matmul(
                        ph[:, 0:BL],
                        fdwt_sb[:, kc * H + m * 128 : kc * H + (m + 1) * 128],
                        docs_v[:, kc, :],
                        start=(kc == 0),
                        stop=(kc == 3),
                    )
                nc.scalar.activation(
                    h_sb[:, m * BL : (m + 1) * BL],
                    ph[:, 0:BL],
                    AF.Relu,
                    bias=fdb_sb[:, m : m + 1],
                )
            po = dd_ps.tile([1, BL], F32, name="sdd")
            for m in range(MH):
                nc.tensor.matmul(
                    po[:],
                    ffwt_sb[:, m : m + 1],
                    h_sb[:, m * BL : (m + 1) * BL],
                    start=(m == 0),
                    stop=(m == MH - 1),
                )
            # sigmoid(x) = 1/(1+exp(-x)) — stays in the Exp table set
            sig_row = hpool.tile([1, BL], F32)
            nc.scalar.activation(sig_row[:], po[:], AF.Exp,
                                 bias=nffb_sb[0:1, 0:1], scale=-1.0)
            nc.vector.tensor_scalar_add(sig_row[:], sig_row[:], 1.0)
            nc.vector.reciprocal(sig_row[:], sig_row[:])
            # final output column: out[:, S] = sigmoid head values
            nc.sync.dma_start(
                out[:, S : S + 1],
                sig_row[0:1, :].rearrange("o (b s) -> o b s", b=BL),
            )


_NC_CACHE = None


def _get_nc():
    global _NC_CACHE
    if _NC_CACHE is None:
        nc = bacc.Bacc("TRN2", target_bir_lowering=False, debug=False,
                       num_devices=NCORES)
        with tile.TileContext(nc) as tc:
            _build_kernel(tc)
        nc.compile()
        _NC_CACHE = nc
    return _NC_CACHE


def kernel(output_1, output_2, mlp_w, mlp_b, fd_w, fd_b, ff_w, ff_b):
    output_1 = np.asarray(output_1, dtype=np.float32)
    output_2 = np.asarray(output_2, dtype=np.float32)
    mlp_w = np.asarray(mlp_w, dtype=np.float32)
    mlp_b = np.asarray(mlp_b, dtype=np.float32)
    fd_w = np.asarray(fd_w, dtype=np.float32)
    fd_b = np.asarray(fd_b, dtype=np.float32)
    ff_w = np.asarray(ff_w, dtype=np.float32)
    ff_b = np.asarray(ff_b, dtype=np.float32)

    # shard over batch; pre-transpose so V lands on partitions with each
    # partition's free row contiguous in HBM
    # x1 e3m4 part (V-chunks 0..3): [c,b,p,k,s] = X1[c*BL+b, s, k*128+p]*SX1
    x1q = np.ascontiguousarray(
        output_1[:, :, : KV3 * 128]
        .reshape(NCORES, BL, S, KV3, 128).transpose(0, 1, 4, 3, 2)
        * SX1
    ).astype(ml_dtypes.float8_e3m4).reshape(NCORES, BL, 128, KV3 * S)
    # x1 e4m3 DR pair (V-chunks 4,5): [c,b,p,i,s]
    x1e = np.ascontiguousarray(
        output_1[:, :, KV3 * 128 :]
        .reshape(NCORES, BL, S, 2, 128).transpose(0, 1, 4, 3, 2)
        * SX1E
    ).astype(ml_dtypes.float8_e4m3).reshape(NCORES, BL, 128, 2 * S)
    # x2[c,b,p,kd,i,s] = X2[c*BL+b, s, kd*256+i*128+p] * SX2, e4m3;
    # x1's e4m3 pair is appended per example so it rides the same DMAs
    x2q = np.ascontiguousarray(
        output_2.reshape(NCORES, BL, S, KD, 2, 128).transpose(0, 1, 5, 3, 4, 2)
        * SX2
    ).astype(ml_dtypes.float8_e4m3).reshape(NCORES, BL, 128, KD * 2 * S)
    x2q = np.ascontiguousarray(np.concatenate([x2q, x1e], axis=3))

    wt = np.ascontiguousarray(mlp_w.T)                    # [V, H] f32
    # w1[p,k,h] = wt[k*128+p, h] * SW1, e3m4 (V rows 0..511)
    w1q = np.ascontiguousarray(
        wt[: KV3 * 128].reshape(KV3, 128, H).transpose(1, 0, 2) * SW1
    ).astype(ml_dtypes.float8_e3m4).reshape(128, KV3 * H)
    # w1e[p,m,i,c] = wt[512 + i*128+p, m*128+c] * SW1E, e4m3
    w1eq = np.ascontiguousarray(
        wt[KV3 * 128 :].reshape(2, 128, MH, 128).transpose(1, 2, 0, 3) * SW1E
    ).astype(ml_dtypes.float8_e4m3).reshape(128, MH * 2 * 128)
    # w2[p,kd,m,i,c] = wt[kd*256+i*128+p, m*128+c] * SW2, e4m3
    w2q = np.ascontiguousarray(
        wt.reshape(KD, 2, 128, MH, 128).transpose(2, 0, 3, 1, 4) * SW2
    ).astype(ml_dtypes.float8_e4m3).reshape(128, KD * MH * 2 * 128)

    mlpb = np.ascontiguousarray(mlp_b.reshape(H, 1))
    fdwt = np.ascontiguousarray(fd_w.T).astype(np.float16)  # [2H, H]
    fdb = np.ascontiguousarray(fd_b.reshape(H, 1))
    ffwt = np.ascontiguousarray(ff_w.T)                   # [H, 1]
    ffb = np.ascontiguousarray(ff_b.reshape(1, 1))

    in_maps = [
        dict(x1t=x1q[c], x2t=x2q[c], w1=w1q, w1e=w1eq, w2=w2q, mlp_b=mlpb,
             fdwt=fdwt, fd_b=fdb, ffwt=ffwt, ff_b=ffb)
        for c in range(NCORES)
    ]
    global _LAST_IN_MAPS
    _LAST_IN_MAPS = in_maps
    nc = _get_nc()
    res = bass_utils.run_bass_kernel_spmd(nc, in_maps, core_ids=list(range(NCORES)))
    att = np.concatenate([res.results[c]["out"] for c in range(NCORES)], axis=0)
    return np.ascontiguousarray(att.T)  # [S+1, B]
